# revision 16
# baseline (speedup 1.0000x reference)
"""MultiHeadAttention kernel for 8 trn2 NeuronCores (Bass/Tile).

Problem: B=2, S=2048, E=1024, H=16, D=64 (fp32), boolean mask [B,S,S].
  out = softmax(mask((q W_q^T) (k W_k^T)^T / sqrt(D))) (v W_v^T) W_o^T + b_o

Sharding: batch x head-group. Core c (c = 4*g + r) handles batch g and heads
4r..4r+3. Per core:
  - QKV projections for its 4 heads (fp16 matmuls, fp32 PSUM accumulate);
    inputs arrive pre-transposed and pre-converted to fp16 by the host
  - attention in transposed layout (scores.T = [k_tok, q_tok]): PE QK with
    2-head row packing, ACT exp straight out of PSUM, DVE mask multiply
    (fp16, 2x mode), PE AV with a fused all-ones stationary column so PSUM
    row 64 accumulates the softmax denominator in the same pass
  - per-(head,q-block) epilogue: DVE reciprocal of the denominator row, PE
    broadcast of it across 64 partitions, DVE multiply -> fp16 av
  - after each q-block: 4-rank AllGather (within the batch group); the
    staging layout splits q into 4 token-slices so phase C needs only a
    single dynamic row offset
  - O-projection is chunked: this core projects token-slice #rank of EVERY
    q-block, so chunks j=0,1 overlap AllGathers 2,3 and the tail after the
    last AllGather is one small chunk
Host side does pure layout marshalling (transpose/slice/cast/concat).
"""

import sys

sys.path.insert(0, "/opt/trn_rl_repo")

import numpy as np
import concourse.bass as bass
import concourse.mybir as mybir
from concourse import bass_types
from concourse.tile import TileContext
from concourse import bass_utils

F32 = mybir.dt.float32
F16 = mybir.dt.float16
I32 = mybir.dt.int32
AF = mybir.ActivationFunctionType
ALU = mybir.AluOpType

P = 128
E = 1024
HPC = 4  # heads per core
EC = HPC * 64  # e_out columns per core (256)
GROUPS = [[0, 1, 2, 3], [4, 5, 6, 7]]

# walrus limits sync-wait commands per instruction (fp32-class matmuls: 1).
# Split excess waits onto NoOps inserted just before, same engine.
_wait_counter = [0]


def _fix_bir_waits(raw: bytes) -> bytes:
    import orjson

    m = orjson.loads(raw)
    for fn in m["functions"]:
        for blk in fn["blocks"]:
            out = []
            changed = False
            for inst in blk["instructions"]:
                si = inst.get("sync_info") or {}
                waits = si.get("on_wait") or []
                if len(waits) > 1:
                    for w in waits[:-1]:
                        _wait_counter[0] += 1
                        out.append(
                            {
                                "engine": inst["engine"],
                                "ins": [],
                                "name": f"I-waitfix-{_wait_counter[0]}",
                                "opcode": "NoOp",
                                "outs": [],
                                "sync_info": {"on_update": [], "on_wait": [w]},
                            }
                        )
                    si["on_wait"] = waits[-1:]
                    inst["sync_info"] = si
                    changed = True
                out.append(inst)
            if changed:
                blk["instructions"] = out
    return orjson.dumps(m)


def build(S: int = 2048) -> bass.Bass:
    KC = S // 128  # k-chunks
    QBW = S // 4  # q-block width (tokens per attention block)
    NQB = 4
    NS = min(512, S)  # projection moving chunk
    TSL = QBW // 4  # token-slice width for chunked O-projection (128)

    nc = bass.Bass()

    xqT = nc.declare_dram_parameter("xqT", [E, S], F16, isOutput=False)
    xkT = nc.declare_dram_parameter("xkT", [E, S], F16, isOutput=False)
    xvT = nc.declare_dram_parameter("xvT", [E, S], F16, isOutput=False)
    maskT = nc.declare_dram_parameter("maskT", [S, S], F16, isOutput=False)
    WqT = nc.declare_dram_parameter("WqT", [E, EC], F16, isOutput=False)
    WkT = nc.declare_dram_parameter("WkT", [E, EC], F16, isOutput=False)
    WvT = nc.declare_dram_parameter("WvT", [E, EC], F16, isOutput=False)
    WoT = nc.declare_dram_parameter("WoT", [E, E], F16, isOutput=False)
    bq = nc.declare_dram_parameter("bq", [EC], F32, isOutput=False)
    bk = nc.declare_dram_parameter("bk", [EC], F32, isOutput=False)
    bv_b = nc.declare_dram_parameter("bv_b", [P, EC], F16, isOutput=False)
    bo_b = nc.declare_dram_parameter("bo_b", [P, E], F32, isOutput=False)
    out = nc.declare_dram_parameter("out", [NQB * TSL, E], F32, isOutput=True)

    with TileContext(nc) as tc:
        with (
            tc.tile_pool(name="persist", bufs=1) as pp,
            tc.tile_pool(name="dramp", bufs=1, space="DRAM") as dramp,
        ):
            # staging: [qb][token-slice 4][2P rows][TSL] -> gathered rank-major
            ag_in = dramp.tile([NQB, 4, 2 * P, TSL], F16)
            ag_out = dramp.tile([NQB * 4 * 4 * 2 * P // P * P, TSL], F16)
            # ag_out rows: qb*4096 + rank*1024 + slice*256 + half*128 + p

            qT_sb = pp.tile([P, 2, S], F16)  # [:, m, :] = q.T rows 128m..128m+127
            kT_sb = pp.tile([P, 2, S], F16)
            # v with 64 fused ones columns per head: [:, t, hh, 0:64] = v
            # rows, [:, t, hh, 64:128] = 1.0, so the AV matmul replicates the
            # softmax denominator across PSUM rows 64..127 for free
            v_sb = pp.tile([P, KC, HPC, P], F16)
            nc.vector.memset(v_sb[:, :, :, 64:P], 1.0)
            bq_sb = pp.tile([P, 2], F32)
            bk_sb = pp.tile([P, 2], F32)
            nc.sync.dma_start(bq_sb[:], bq.rearrange("(m p) -> p m", p=P))
            nc.sync.dma_start(bk_sb[:], bk.rearrange("(m p) -> p m", p=P))
            bv_sb = pp.tile([P, EC], F16)
            nc.gpsimd.dma_start(bv_sb[:], bv_b[:])
            bo_sb = pp.tile([P, E], F32)
            nc.sync.dma_start(bo_sb[:], bo_b[:])

            rank = nc.gpsimd.snap(
                nc.gpsimd.cc_rank(replica_groups=GROUPS), min_val=0, max_val=3
            )

            # ---------------- Phase A: QKV projections (order k, q, v) -------
            with (
                tc.tile_pool(name="wpool", bufs=1) as wp,
                tc.tile_pool(name="xpool", bufs=3) as xp,
                tc.tile_pool(name="psA", bufs=8, space="PSUM") as psA,
            ):
                wq_sb = wp.tile([P, 8, EC], F16)
                wk_sb = wp.tile([P, 8, EC], F16)
                wv_sb = wp.tile([P, 8, EC], F16)
                nc.gpsimd.dma_start(wk_sb[:], WkT.rearrange("(kt p) m -> p kt m", p=P))
                nc.gpsimd.dma_start(wq_sb[:], WqT.rearrange("(kt p) m -> p kt m", p=P))
                nc.gpsimd.dma_start(wv_sb[:], WvT.rearrange("(kt p) m -> p kt m", p=P))

                for which in range(3):
                    xT, w_sb = [(xkT, wk_sb), (xqT, wq_sb), (xvT, wv_sb)][which]
                    nps = (2 * S) // NS if which < 2 else KC // 2
                    pst = [
                        psA.tile([P, 512], F32, name=f"psA_{which}_{i}", tag="psA")
                        for i in range(nps)
                    ]
                    for kt in range(8):
                        x_t = xp.tile([P, S], F16, name=f"x_{which}_{kt}", tag="x")
                        x_dma = nc.sync.dma_start(x_t[:], xT[kt * P : (kt + 1) * P, :])
                        if which == 2 and kt == 7:
                            last_x_dma = x_dma
                        if which < 2:
                            # k.T / q.T: out [256, S]; lhsT = W tile, rhs = x.T
                            for m in range(2):
                                lhsT = w_sb[:, kt, m * P : (m + 1) * P]
                                for n in range(S // NS):
                                    nc.tensor.matmul(
                                        pst[m * (S // NS) + n][:, :NS],
                                        lhsT,
                                        x_t[:, n * NS : (n + 1) * NS],
                                        start=(kt == 0),
                                        stop=(kt == 7),
                                    )
                        else:
                            # v: out [S, 256]; lhsT = x.T tile, rhs = W k-tile.
                            # Two token-chunks share one PSUM bank: the
                            # has_written group opens on the even chunk and
                            # closes on the odd one (2KB zero-region rule).
                            for t in range(KC):
                                nc.tensor.matmul(
                                    pst[t // 2][:, (t % 2) * EC : (t % 2 + 1) * EC],
                                    x_t[:, t * P : (t + 1) * P],
                                    w_sb[:, kt, :],
                                    start=(kt == 0 and t % 2 == 0),
                                    stop=(kt == 7 and t % 2 == 1),
                                )
                    if which == 0:
                        for m in range(2):
                            for n in range(S // NS):
                                nc.vector.tensor_scalar(
                                    kT_sb[:, m, n * NS : (n + 1) * NS],
                                    pst[m * (S // NS) + n][:, :NS],
                                    1.0,
                                    bk_sb[:, m : m + 1],
                                    ALU.mult,
                                    ALU.add,
                                )
                    elif which == 1:
                        for m in range(2):
                            for n in range(S // NS):
                                # (q + bq) / 8, bias before scale
                                nc.vector.tensor_scalar(
                                    qT_sb[:, m, n * NS : (n + 1) * NS],
                                    pst[m * (S // NS) + n][:, :NS],
                                    bq_sb[:, m : m + 1],
                                    0.125,
                                    ALU.add,
                                    ALU.mult,
                                )
                    else:
                        for t in range(KC):
                            nc.vector.tensor_tensor(
                                v_sb[:, t, :, 0:64],
                                pst[t // 2][
                                    :, (t % 2) * EC : (t % 2 + 1) * EC
                                ].rearrange("p (h d) -> p h d", h=HPC),
                                bv_sb[:].rearrange("p (h d) -> p h d", h=HPC),
                                ALU.add,
                            )

            # -------- Phase B: attention + per-qb AllGather + chunked O-proj -
            with (
                tc.tile_pool(name="maskpool", bufs=1) as mp,
                tc.tile_pool(name="ppool", bufs=3) as ppl,
                tc.tile_pool(name="epool", bufs=4) as ep,
                tc.tile_pool(name="cpool", bufs=1) as cp,
                tc.tile_pool(name="atpool", bufs=2) as atp,
                tc.tile_pool(name="opool", bufs=2) as op,
                tc.tile_pool(name="sps", bufs=2, space="PSUM") as sps,
                tc.tile_pool(name="avps", bufs=2, space="PSUM") as avps,
                tc.tile_pool(name="ops", bufs=2, space="PSUM") as ops,
            ):
                from concourse.tile_rust import add_dep_helper

                maskbf = mp.tile([P, KC, S], F16)
                for t in range(KC):
                    mdma = nc.gpsimd.dma_start(
                        maskbf[:, t, :], maskT[t * P : (t + 1) * P, :]
                    )
                    if t == 0:
                        # keep the big mask stream off phase A's DMA window
                        add_dep_helper(
                            mdma.ins,
                            last_x_dma.ins,
                            reason="defer mask load until x loads finish",
                        )

                woT_sb = cp.tile([P, 8, E], F16)
                wo_dma = nc.gpsimd.dma_start(
                    woT_sb[:], WoT.rearrange("(kt p) n -> p kt n", p=P)
                )
                add_dep_helper(
                    wo_dma.ins,
                    last_x_dma.ins,
                    reason="defer WoT load off phase A's DMA window",
                )

                def phase_c_chunk(j):
                    # project token-slice #rank of q-block j (tokens
                    # j*QBW + rank*TSL ..+TSL) -> out rows j*TSL..
                    # attnT[:, h, r', :] = E-rows r'*256+h*128.. of gather
                    # rank r'; one 3-dim dynamic DMA per h half (the Pool
                    # engine has a small budget of dynamic-DMA registers)
                    attnT = atp.tile([P, 2, 4, TSL], F16, name=f"at_{j}", tag="at")
                    for h in range(2):
                        base = ag_out[
                            bass.ds(j * 4096 + rank * 256 + h * P, P), :
                        ]
                        manual = bass_types.AP(
                            base.tensor,
                            base.offset,
                            [[TSL, P], [1024 * TSL, 4], [1, TSL]],
                        )
                        nc.gpsimd.dma_start(attnT[:, h, :, :], manual)
                    for n in range(2):
                        o_ps = ops.tile([P, 512], F32, name=f"o_{j}_{n}", tag="o")
                        for kt in range(8):
                            nc.tensor.matmul(
                                o_ps[:, :],
                                attnT[:, kt % 2, kt // 2, :],
                                woT_sb[:, kt, n * 512 : (n + 1) * 512],
                                start=(kt == 0),
                                stop=(kt == 7),
                            )
                        out_sb = op.tile(
                            [P, 512], F32, name=f"osb_{j}_{n}", tag="osb"
                        )
                        nc.vector.tensor_tensor(
                            out_sb[:, :],
                            o_ps[:, :],
                            bo_sb[:, n * 512 : (n + 1) * 512],
                            ALU.add,
                        )
                        nc.sync.dma_start(
                            out[j * TSL : (j + 1) * TSL, n * 512 : (n + 1) * 512],
                            out_sb[:, :],
                        )

                for qb in range(NQB):
                    qsl = slice(qb * QBW, (qb + 1) * QBW)
                    for pair in range(2):
                        av_t = [
                            avps.tile(
                                [P, QBW], F32, name=f"av_{qb}_{pair}_{h}", tag="av"
                            )
                            for h in range(2)
                        ]
                        for kc in range(KC):
                            ksl = slice(kc * P, (kc + 1) * P)
                            s_t = sps.tile(
                                [P, 2, QBW], F32, name=f"s_{qb}_{pair}_{kc}", tag="s"
                            )
                            for h in range(2):
                                prt = slice(h * 64, (h + 1) * 64)
                                nc.tensor.matmul(
                                    s_t[:, h, :],
                                    kT_sb[prt, pair, ksl],
                                    qT_sb[prt, pair, qsl],
                                    start=True,
                                    stop=True,
                                )
                            p_t = ppl.tile([P, 2 * QBW], F16, name="p_t", tag="p")
                            nc.scalar.activation(
                                p_t[:].rearrange("p (h n) -> p h n", h=2),
                                s_t[:, :, :],
                                AF.Exp,
                            )
                            for h in range(2):
                                # dense fp16 ops (no broadcast) keep DVE in
                                # its 2x perf mode; h=1 goes to the otherwise
                                # idle GpSimd engine to split the load
                                eng = nc.vector if h == 0 else nc.gpsimd
                                eng.tensor_tensor(
                                    p_t[:, h * QBW : (h + 1) * QBW],
                                    p_t[:, h * QBW : (h + 1) * QBW],
                                    maskbf[:, kc, qsl],
                                    ALU.mult,
                                )
                            for h in range(2):
                                hsl = slice(h * QBW, (h + 1) * QBW)
                                # fused [v | ones] stationary: PSUM rows 0..63
                                # accumulate AV, rows 64..127 the denominator
                                nc.tensor.matmul(
                                    av_t[h][:, :],
                                    v_sb[:, kc, pair * 2 + h, :],
                                    p_t[:, hsl],
                                    start=(kc == 0),
                                    stop=(kc == KC - 1),
                                )
                        # epilogue for this pair: divide + stage
                        for h in range(2):
                            rb = ep.tile([64, QBW], F16, name="rb", tag="rb")
                            with nc.allow_low_precision(
                                reason="fp16 reciprocal of softmax denom; "
                                "5e-4 rel err is within the 2e-2 gate"
                            ):
                                nc.vector.reciprocal(rb[:], av_t[h][64:P, :])
                            av_f = ep.tile([64, QBW], F16, name="av_f", tag="av_f")
                            nc.vector.tensor_tensor(
                                av_f[:, :],
                                av_t[h][0:64, :],
                                rb[:, :],
                                ALU.mult,
                            )
                            for sl in range(4):
                                nc.sync.dma_start(
                                    ag_in[
                                        qb,
                                        sl,
                                        pair * P + h * 64 : pair * P + (h + 1) * 64,
                                        :,
                                    ],
                                    av_f[:, sl * TSL : (sl + 1) * TSL],
                                )
                    nc.gpsimd.collective_compute(
                        "AllGather",
                        ALU.bypass,
                        ins=[ag_in[qb]],
                        outs=[ag_out[qb * 4096 : (qb + 1) * 4096, :]],
                        replica_groups=GROUPS,
                    )
                    if qb >= 2:
                        phase_c_chunk(qb - 2)
                phase_c_chunk(2)
                phase_c_chunk(3)

    fixed = _fix_bir_waits(nc.to_json_bytes())
    nc.to_json_bytes = lambda: fixed
    return nc


_NC_CACHE: dict = {}


def _get_nc(S: int) -> bass.Bass:
    if S not in _NC_CACHE:
        _NC_CACHE[S] = build(S)
    return _NC_CACHE[S]


def kernel(
    query,
    key,
    value,
    mask,
    Wq,
    bq,
    Wk,
    bk,
    Wv,
    bv,
    Wo,
    bo,
    _trace: bool = False,
    _trace_dir: str | None = None,
):
    query = np.asarray(query, np.float32)
    key = np.asarray(key, np.float32)
    value = np.asarray(value, np.float32)
    mask = np.asarray(mask, np.int32)
    Wq = np.asarray(Wq, np.float32)
    Wk = np.asarray(Wk, np.float32)
    Wv = np.asarray(Wv, np.float32)
    Wo = np.asarray(Wo, np.float32)
    bq = np.asarray(bq, np.float32)
    bk = np.asarray(bk, np.float32)
    bv = np.asarray(bv, np.float32)
    bo = np.asarray(bo, np.float32)

    B, S, E_ = query.shape
    assert (B, E_) == (2, 1024), (B, E_)
    nc = _get_nc(S)

    # host-side layout marshalling (transpose/cast only, no arithmetic)
    xT = {}
    for g in range(2):
        xT[("q", g)] = np.ascontiguousarray(query[g].T.astype(np.float16))
        xT[("k", g)] = np.ascontiguousarray(key[g].T.astype(np.float16))
        xT[("v", g)] = np.ascontiguousarray(value[g].T.astype(np.float16))
    maskTt = [np.ascontiguousarray(mask[g].T.astype(np.float16)) for g in range(2)]
    WoT_h = np.ascontiguousarray(Wo.T.astype(np.float16))
    bo_rep = np.ascontiguousarray(np.broadcast_to(bo, (128, 1024)))

    in_maps = []
    for c in range(8):
        g, r = divmod(c, 4)
        hs = slice(r * EC, (r + 1) * EC)
        in_maps.append(
            {
                "xqT": xT[("q", g)],
                "xkT": xT[("k", g)],
                "xvT": xT[("v", g)],
                "maskT": maskTt[g],
                "WqT": np.ascontiguousarray(Wq[hs, :].T.astype(np.float16)),
                "WkT": np.ascontiguousarray(Wk[hs, :].T.astype(np.float16)),
                "WvT": np.ascontiguousarray(Wv[hs, :].T.astype(np.float16)),
                "WoT": WoT_h,
                "bq": np.ascontiguousarray(bq[hs]),
                "bk": np.ascontiguousarray(bk[hs]),
                "bv_b": np.ascontiguousarray(
                    np.broadcast_to(bv[hs].astype(np.float16), (128, EC))
                ),
                "bo_b": bo_rep,
            }
        )

    kw = {}
    if _trace:
        kw = dict(trace=True, tmpdir=_trace_dir)
    res = bass_utils.run_bass_kernel_spmd(nc, in_maps, list(range(8)), **kw)

    QBW = S // 4
    TSL = QBW // 4
    out_full = np.empty((B, S, E_), np.float32)
    for c in range(8):
        g, r = divmod(c, 4)
        o = res.results[c]["out"]
        for j in range(4):
            out_full[g, j * QBW + r * TSL : j * QBW + (r + 1) * TSL, :] = o[
                j * TSL : (j + 1) * TSL, :
            ]
    if _trace:
        kernel._last_exec_time_ns = res.exec_time_ns
        kernel._last_trace = res.instructions_and_trace
    return out_full


# revision 18
# speedup vs baseline: 1.1128x; 1.1128x over previous
"""MultiHeadAttention kernel for 8 trn2 NeuronCores (Bass/Tile).

Problem: B=2, S=2048, E=1024, H=16, D=64 (fp32), boolean mask [B,S,S].
  out = softmax(mask((q W_q^T) (k W_k^T)^T / sqrt(D))) (v W_v^T) W_o^T + b_o

Sharding: batch x head-group. Core c (c = 4*g + r) handles batch g and heads
4r..4r+3. Per core:
  - QKV projections for its 4 heads (fp16 matmuls, fp32 PSUM accumulate);
    inputs arrive pre-transposed and pre-converted to fp16 by the host
  - attention in transposed layout (scores.T = [k_tok, q_tok]): PE QK with
    2-head row packing, ACT exp straight out of PSUM, DVE mask multiply
    (fp16, 2x mode), PE AV with a fused all-ones stationary column so PSUM
    row 64 accumulates the softmax denominator in the same pass
  - per-(head,q-block) epilogue: DVE reciprocal of the denominator row, PE
    broadcast of it across 64 partitions, DVE multiply -> fp16 av
  - after each q-block: 4-rank AllGather (within the batch group); the
    staging layout splits q into 4 token-slices so phase C needs only a
    single dynamic row offset
  - O-projection is chunked: this core projects token-slice #rank of EVERY
    q-block, so chunks j=0,1 overlap AllGathers 2,3 and the tail after the
    last AllGather is one small chunk
Host side does pure layout marshalling (transpose/slice/cast/concat).
"""

import sys

sys.path.insert(0, "/opt/trn_rl_repo")

import numpy as np
import concourse.bass as bass
import concourse.mybir as mybir
from concourse import bass_types
from concourse.tile import TileContext
from concourse import bass_utils

F32 = mybir.dt.float32
F16 = mybir.dt.float16
I32 = mybir.dt.int32
U8 = mybir.dt.uint8
AF = mybir.ActivationFunctionType
ALU = mybir.AluOpType

P = 128
E = 1024
HPC = 4  # heads per core
EC = HPC * 64  # e_out columns per core (256)
GROUPS = [[0, 1, 2, 3], [4, 5, 6, 7]]

# walrus limits sync-wait commands per instruction (fp32-class matmuls: 1).
# Split excess waits onto NoOps inserted just before, same engine.
_wait_counter = [0]


def _fix_bir_waits(raw: bytes) -> bytes:
    import orjson

    m = orjson.loads(raw)
    for fn in m["functions"]:
        for blk in fn["blocks"]:
            out = []
            changed = False
            for inst in blk["instructions"]:
                si = inst.get("sync_info") or {}
                waits = si.get("on_wait") or []
                if len(waits) > 1:
                    for w in waits[:-1]:
                        _wait_counter[0] += 1
                        out.append(
                            {
                                "engine": inst["engine"],
                                "ins": [],
                                "name": f"I-waitfix-{_wait_counter[0]}",
                                "opcode": "NoOp",
                                "outs": [],
                                "sync_info": {"on_update": [], "on_wait": [w]},
                            }
                        )
                    si["on_wait"] = waits[-1:]
                    inst["sync_info"] = si
                    changed = True
                out.append(inst)
            if changed:
                blk["instructions"] = out
    return orjson.dumps(m)


def build(S: int = 2048) -> bass.Bass:
    KC = S // 128  # k-chunks
    QBW = S // 4  # q-block width (tokens per attention block)
    NQB = 4
    NS = min(512, S)  # projection moving chunk
    TSL = QBW // 4  # token-slice width for chunked O-projection (128)

    nc = bass.Bass()

    xqT = nc.declare_dram_parameter("xqT", [E, S], F16, isOutput=False)
    xkT = nc.declare_dram_parameter("xkT", [E, S], F16, isOutput=False)
    xvT = nc.declare_dram_parameter("xvT", [E, S], F16, isOutput=False)
    maskT = nc.declare_dram_parameter("maskT", [S, S], U8, isOutput=False)
    WqT = nc.declare_dram_parameter("WqT", [E, EC], F16, isOutput=False)
    WkT = nc.declare_dram_parameter("WkT", [E, EC], F16, isOutput=False)
    WvT = nc.declare_dram_parameter("WvT", [E, EC], F16, isOutput=False)
    WoT = nc.declare_dram_parameter("WoT", [E, E], F16, isOutput=False)
    bq = nc.declare_dram_parameter("bq", [EC], F32, isOutput=False)
    bk = nc.declare_dram_parameter("bk", [EC], F32, isOutput=False)
    bv_b = nc.declare_dram_parameter("bv_b", [P, EC], F16, isOutput=False)
    bo_b = nc.declare_dram_parameter("bo_b", [P, E], F32, isOutput=False)
    out = nc.declare_dram_parameter("out", [NQB * TSL, E], F32, isOutput=True)

    with TileContext(nc) as tc:
        with (
            tc.tile_pool(name="persist", bufs=1) as pp,
            tc.tile_pool(name="dramp", bufs=1, space="DRAM") as dramp,
        ):
            # staging: [qb][token-slice 4][2P rows][TSL] -> gathered rank-major
            ag_in = dramp.tile([NQB, 4, 2 * P, TSL], F16)
            ag_out = dramp.tile([NQB * 4 * 4 * 2 * P // P * P, TSL], F16)
            # ag_out rows: qb*4096 + rank*1024 + slice*256 + half*128 + p

            qT_sb = pp.tile([P, 2, S], F16)  # [:, m, :] = q.T rows 128m..128m+127
            kT_sb = pp.tile([P, 2, S], F16)
            # v with 64 fused ones columns per head: [:, t, hh, 0:64] = v
            # rows, [:, t, hh, 64:128] = 1.0, so the AV matmul replicates the
            # softmax denominator across PSUM rows 64..127 for free
            v_sb = pp.tile([P, KC, HPC, P], F16)
            nc.vector.memset(v_sb[:, :, :, 64:P], 1.0)
            bq_sb = pp.tile([P, 2], F32)
            bk_sb = pp.tile([P, 2], F32)
            nc.sync.dma_start(bq_sb[:], bq.rearrange("(m p) -> p m", p=P))
            nc.sync.dma_start(bk_sb[:], bk.rearrange("(m p) -> p m", p=P))
            bv_sb = pp.tile([P, EC], F16)
            nc.gpsimd.dma_start(bv_sb[:], bv_b[:])
            bo_sb = pp.tile([P, E], F32)
            nc.sync.dma_start(bo_sb[:], bo_b[:])

            rank = nc.gpsimd.snap(
                nc.gpsimd.cc_rank(replica_groups=GROUPS), min_val=0, max_val=3
            )

            # ---------------- Phase A: QKV projections (order k, q, v) -------
            with (
                tc.tile_pool(name="wpool", bufs=1) as wp,
                tc.tile_pool(name="xpool", bufs=3) as xp,
                tc.tile_pool(name="psA", bufs=8, space="PSUM") as psA,
            ):
                wq_sb = wp.tile([P, 8, EC], F16)
                wk_sb = wp.tile([P, 8, EC], F16)
                wv_sb = wp.tile([P, 8, EC], F16)
                nc.gpsimd.dma_start(wk_sb[:], WkT.rearrange("(kt p) m -> p kt m", p=P))
                nc.gpsimd.dma_start(wq_sb[:], WqT.rearrange("(kt p) m -> p kt m", p=P))
                nc.gpsimd.dma_start(wv_sb[:], WvT.rearrange("(kt p) m -> p kt m", p=P))

                for which in range(3):
                    xT, w_sb = [(xkT, wk_sb), (xqT, wq_sb), (xvT, wv_sb)][which]
                    nps = (2 * S) // NS if which < 2 else KC // 2
                    pst = [
                        psA.tile([P, 512], F32, name=f"psA_{which}_{i}", tag="psA")
                        for i in range(nps)
                    ]
                    for kt in range(8):
                        x_t = xp.tile([P, S], F16, name=f"x_{which}_{kt}", tag="x")
                        x_dma = nc.sync.dma_start(x_t[:], xT[kt * P : (kt + 1) * P, :])
                        if which == 2 and kt == 7:
                            last_x_dma = x_dma
                        if which < 2:
                            # k.T / q.T: out [256, S]; lhsT = W tile, rhs = x.T
                            for m in range(2):
                                lhsT = w_sb[:, kt, m * P : (m + 1) * P]
                                for n in range(S // NS):
                                    nc.tensor.matmul(
                                        pst[m * (S // NS) + n][:, :NS],
                                        lhsT,
                                        x_t[:, n * NS : (n + 1) * NS],
                                        start=(kt == 0),
                                        stop=(kt == 7),
                                    )
                        else:
                            # v: out [S, 256]; lhsT = x.T tile, rhs = W k-tile.
                            # Two token-chunks share one PSUM bank: the
                            # has_written group opens on the even chunk and
                            # closes on the odd one (2KB zero-region rule).
                            for t in range(KC):
                                nc.tensor.matmul(
                                    pst[t // 2][:, (t % 2) * EC : (t % 2 + 1) * EC],
                                    x_t[:, t * P : (t + 1) * P],
                                    w_sb[:, kt, :],
                                    start=(kt == 0 and t % 2 == 0),
                                    stop=(kt == 7 and t % 2 == 1),
                                )
                    if which == 0:
                        for m in range(2):
                            for n in range(S // NS):
                                nc.vector.tensor_scalar(
                                    kT_sb[:, m, n * NS : (n + 1) * NS],
                                    pst[m * (S // NS) + n][:, :NS],
                                    1.0,
                                    bk_sb[:, m : m + 1],
                                    ALU.mult,
                                    ALU.add,
                                )
                    elif which == 1:
                        for m in range(2):
                            for n in range(S // NS):
                                # (q + bq) / 8, bias before scale
                                nc.vector.tensor_scalar(
                                    qT_sb[:, m, n * NS : (n + 1) * NS],
                                    pst[m * (S // NS) + n][:, :NS],
                                    bq_sb[:, m : m + 1],
                                    0.125,
                                    ALU.add,
                                    ALU.mult,
                                )
                    else:
                        for t in range(KC):
                            nc.vector.tensor_tensor(
                                v_sb[:, t, :, 0:64],
                                pst[t // 2][
                                    :, (t % 2) * EC : (t % 2 + 1) * EC
                                ].rearrange("p (h d) -> p h d", h=HPC),
                                bv_sb[:].rearrange("p (h d) -> p h d", h=HPC),
                                ALU.add,
                            )

            # -------- Phase B: attention + per-qb AllGather + chunked O-proj -
            with (
                tc.tile_pool(name="maskpool", bufs=1) as mp,
                tc.tile_pool(name="ppool", bufs=3) as ppl,
                tc.tile_pool(name="epool", bufs=4) as ep,
                tc.tile_pool(name="cpool", bufs=1) as cp,
                tc.tile_pool(name="atpool", bufs=2) as atp,
                tc.tile_pool(name="opool", bufs=2) as op,
                tc.tile_pool(name="sps", bufs=2, space="PSUM") as sps,
                tc.tile_pool(name="avps", bufs=2, space="PSUM") as avps,
                tc.tile_pool(name="ops", bufs=2, space="PSUM") as ops,
            ):
                from concourse.tile_rust import add_dep_helper

                maskbf = mp.tile([P, KC, S], U8)
                zero_sb = mp.tile([P, QBW], F16, name="zero_sb")
                nc.vector.memset(zero_sb[:], 0.0)
                for t in range(KC):
                    mdma = nc.gpsimd.dma_start(
                        maskbf[:, t, :], maskT[t * P : (t + 1) * P, :]
                    )
                    if t == 0:
                        # keep the big mask stream off phase A's DMA window
                        add_dep_helper(
                            mdma.ins,
                            last_x_dma.ins,
                            reason="defer mask load until x loads finish",
                        )

                woT_sb = cp.tile([P, 8, E], F16)
                wo_dma = nc.gpsimd.dma_start(
                    woT_sb[:], WoT.rearrange("(kt p) n -> p kt n", p=P)
                )
                add_dep_helper(
                    wo_dma.ins,
                    last_x_dma.ins,
                    reason="defer WoT load off phase A's DMA window",
                )

                def phase_c_chunk(j):
                    # project token-slice #rank of q-block j (tokens
                    # j*QBW + rank*TSL ..+TSL) -> out rows j*TSL..
                    # attnT[:, h, r', :] = E-rows r'*256+h*128.. of gather
                    # rank r'; one 3-dim dynamic DMA per h half (the Pool
                    # engine has a small budget of dynamic-DMA registers)
                    attnT = atp.tile([P, 2, 4, TSL], F16, name=f"at_{j}", tag="at")
                    for h in range(2):
                        base = ag_out[
                            bass.ds(j * 4096 + rank * 256 + h * P, P), :
                        ]
                        manual = bass_types.AP(
                            base.tensor,
                            base.offset,
                            [[TSL, P], [1024 * TSL, 4], [1, TSL]],
                        )
                        nc.gpsimd.dma_start(attnT[:, h, :, :], manual)
                    for n in range(2):
                        o_ps = ops.tile([P, 512], F32, name=f"o_{j}_{n}", tag="o")
                        for kt in range(8):
                            nc.tensor.matmul(
                                o_ps[:, :],
                                attnT[:, kt % 2, kt // 2, :],
                                woT_sb[:, kt, n * 512 : (n + 1) * 512],
                                start=(kt == 0),
                                stop=(kt == 7),
                            )
                        out_sb = op.tile(
                            [P, 512], F32, name=f"osb_{j}_{n}", tag="osb"
                        )
                        nc.vector.tensor_tensor(
                            out_sb[:, :],
                            o_ps[:, :],
                            bo_sb[:, n * 512 : (n + 1) * 512],
                            ALU.add,
                        )
                        nc.sync.dma_start(
                            out[j * TSL : (j + 1) * TSL, n * 512 : (n + 1) * 512],
                            out_sb[:, :],
                        )

                for qb in range(NQB):
                    qsl = slice(qb * QBW, (qb + 1) * QBW)
                    for pair in range(2):
                        av_t = [
                            avps.tile(
                                [P, QBW], F32, name=f"av_{qb}_{pair}_{h}", tag="av"
                            )
                            for h in range(2)
                        ]
                        for kc in range(KC):
                            ksl = slice(kc * P, (kc + 1) * P)
                            s_t = sps.tile(
                                [P, 2, QBW], F32, name=f"s_{qb}_{pair}_{kc}", tag="s"
                            )
                            for h in range(2):
                                prt = slice(h * 64, (h + 1) * 64)
                                nc.tensor.matmul(
                                    s_t[:, h, :],
                                    kT_sb[prt, pair, ksl],
                                    qT_sb[prt, pair, qsl],
                                    start=True,
                                    stop=True,
                                )
                            p_t = ppl.tile([P, 2 * QBW], F16, name="p_t", tag="p")
                            nc.scalar.activation(
                                p_t[:].rearrange("p (h n) -> p h n", h=2),
                                s_t[:, :, :],
                                AF.Exp,
                            )
                            for h in range(2):
                                # zero the masked-out positions in place:
                                # maskbf holds the HOST-INVERTED mask (u8,
                                # 1 = masked out), so copy_predicated writes
                                # 0.0 exactly there and keeps exp(s) elsewhere
                                nc.vector.copy_predicated(
                                    p_t[:, h * QBW : (h + 1) * QBW],
                                    maskbf[:, kc, qsl],
                                    zero_sb[:],
                                )
                            for h in range(2):
                                hsl = slice(h * QBW, (h + 1) * QBW)
                                # fused [v | ones] stationary: PSUM rows 0..63
                                # accumulate AV, rows 64..127 the denominator
                                nc.tensor.matmul(
                                    av_t[h][:, :],
                                    v_sb[:, kc, pair * 2 + h, :],
                                    p_t[:, hsl],
                                    start=(kc == 0),
                                    stop=(kc == KC - 1),
                                )
                        # epilogue for this pair: divide + stage
                        for h in range(2):
                            rb = ep.tile([64, QBW], F16, name="rb", tag="rb")
                            with nc.allow_low_precision(
                                reason="fp16 reciprocal of softmax denom; "
                                "5e-4 rel err is within the 2e-2 gate"
                            ):
                                nc.vector.reciprocal(rb[:], av_t[h][64:P, :])
                            av_f = ep.tile([64, QBW], F16, name="av_f", tag="av_f")
                            nc.vector.tensor_tensor(
                                av_f[:, :],
                                av_t[h][0:64, :],
                                rb[:, :],
                                ALU.mult,
                            )
                            for sl in range(4):
                                nc.sync.dma_start(
                                    ag_in[
                                        qb,
                                        sl,
                                        pair * P + h * 64 : pair * P + (h + 1) * 64,
                                        :,
                                    ],
                                    av_f[:, sl * TSL : (sl + 1) * TSL],
                                )
                    nc.gpsimd.collective_compute(
                        "AllGather",
                        ALU.bypass,
                        ins=[ag_in[qb]],
                        outs=[ag_out[qb * 4096 : (qb + 1) * 4096, :]],
                        replica_groups=GROUPS,
                    )
                    if qb >= 2:
                        phase_c_chunk(qb - 2)
                phase_c_chunk(2)
                phase_c_chunk(3)

    fixed = _fix_bir_waits(nc.to_json_bytes())
    nc.to_json_bytes = lambda: fixed
    return nc


_NC_CACHE: dict = {}


def _get_nc(S: int) -> bass.Bass:
    if S not in _NC_CACHE:
        _NC_CACHE[S] = build(S)
    return _NC_CACHE[S]


def kernel(
    query,
    key,
    value,
    mask,
    Wq,
    bq,
    Wk,
    bk,
    Wv,
    bv,
    Wo,
    bo,
    _trace: bool = False,
    _trace_dir: str | None = None,
):
    query = np.asarray(query, np.float32)
    key = np.asarray(key, np.float32)
    value = np.asarray(value, np.float32)
    mask = np.asarray(mask, np.int32)
    Wq = np.asarray(Wq, np.float32)
    Wk = np.asarray(Wk, np.float32)
    Wv = np.asarray(Wv, np.float32)
    Wo = np.asarray(Wo, np.float32)
    bq = np.asarray(bq, np.float32)
    bk = np.asarray(bk, np.float32)
    bv = np.asarray(bv, np.float32)
    bo = np.asarray(bo, np.float32)

    B, S, E_ = query.shape
    assert (B, E_) == (2, 1024), (B, E_)
    nc = _get_nc(S)

    # host-side layout marshalling (transpose/cast only, no arithmetic)
    xT = {}
    for g in range(2):
        xT[("q", g)] = np.ascontiguousarray(query[g].T.astype(np.float16))
        xT[("k", g)] = np.ascontiguousarray(key[g].T.astype(np.float16))
        xT[("v", g)] = np.ascontiguousarray(value[g].T.astype(np.float16))
    maskTt = [np.ascontiguousarray((1 - mask[g].T).astype(np.uint8)) for g in range(2)]
    WoT_h = np.ascontiguousarray(Wo.T.astype(np.float16))
    bo_rep = np.ascontiguousarray(np.broadcast_to(bo, (128, 1024)))

    in_maps = []
    for c in range(8):
        g, r = divmod(c, 4)
        hs = slice(r * EC, (r + 1) * EC)
        in_maps.append(
            {
                "xqT": xT[("q", g)],
                "xkT": xT[("k", g)],
                "xvT": xT[("v", g)],
                "maskT": maskTt[g],
                "WqT": np.ascontiguousarray(Wq[hs, :].T.astype(np.float16)),
                "WkT": np.ascontiguousarray(Wk[hs, :].T.astype(np.float16)),
                "WvT": np.ascontiguousarray(Wv[hs, :].T.astype(np.float16)),
                "WoT": WoT_h,
                "bq": np.ascontiguousarray(bq[hs]),
                "bk": np.ascontiguousarray(bk[hs]),
                "bv_b": np.ascontiguousarray(
                    np.broadcast_to(bv[hs].astype(np.float16), (128, EC))
                ),
                "bo_b": bo_rep,
            }
        )

    kw = {}
    if _trace:
        kw = dict(trace=True, tmpdir=_trace_dir)
    res = bass_utils.run_bass_kernel_spmd(nc, in_maps, list(range(8)), **kw)

    QBW = S // 4
    TSL = QBW // 4
    out_full = np.empty((B, S, E_), np.float32)
    for c in range(8):
        g, r = divmod(c, 4)
        o = res.results[c]["out"]
        for j in range(4):
            out_full[g, j * QBW + r * TSL : j * QBW + (r + 1) * TSL, :] = o[
                j * TSL : (j + 1) * TSL, :
            ]
    if _trace:
        kernel._last_exec_time_ns = res.exec_time_ns
        kernel._last_trace = res.instructions_and_trace
    return out_full
